# revision 6
# baseline (speedup 1.0000x reference)
"""Chamfer L2 distance kernel for 8 Trainium2 NeuronCores.

Shared-distance-matrix design (2 batches/core): one matmul pass per batch
computes the FULL squared distance D[n,m] = |x_n|^2 - 2<x_n,y_m> + |y_m|^2
(K=24 bf16 contraction rows), and BOTH chamfer directions are reduced from
that single pass:

  rowmin[n] = min_m D   -> negated-max accumulators, engine paths per tile:
    * "S" tiles: a hand-wired custom DVE op drains the PSUM chunk,
      negates, row-max-accumulates AND updates the column-max accumulator
      R = max(R, -D) in a single 1 elem/cycle pass.
    * "G"/"D" tiles: ScalarE copies -D to bf16 SBUF; a custom packed-bf16
      2x_1p DVE op (4 elem/cycle) row-max-reduces chunk pairs; the column
      accumulation runs on GPSIMD (partition-axis max reduce, per-tile
      result DMA'd to DRAM) for G tiles or on DVE (bf16 TT-max, 2x_1p,
      ping-pong accumulator) for D tiles.
  colmin[m] = min_n D   -> final folds across accumulators done on host.

Self-contained: hardcodes B=16, N=M=4096, C=3, 8 cores.
"""

import numpy as np
import ml_dtypes

BF = ml_dtypes.bfloat16
B, N, M, C = 16, 4096, 4096, 3
NCORES = 8
BPC = B // NCORES          # batches per core
K = 24                     # contraction rows
NT = N // 128              # n-tiles per batch
CHW = 1024                 # PSUM chunk width
NCH = M // CHW             # chunks per tile row-block
NEG_INIT = -3.0e38

# per-batch tile path pattern; counts tuned against the engine cost model
N_S, N_D, N_G = 12, 0, 20


def _make_pattern():
    counts = {k: v for k, v in (("S", N_S), ("D", N_D), ("G", N_G)) if v}
    total = sum(counts.values())
    assert total == NT
    acc = {k: 0.0 for k in counts}
    out = []
    for _ in range(total):
        for k in counts:
            acc[k] += counts[k] / total
        k = max(acc, key=lambda q: acc[q])
        acc[k] -= 1.0
        out.append(k)
    return out


PATTERN = _make_pattern()
N_A = N_D + N_G                    # ACT tiles per batch
SLOTF = NCH * N_S                  # f32 rowacc slots per batch
SLOTB = 2 * (NCH // 2) * N_A       # bf16 rowacc cols per batch (stride-2)

_CACHE = {}


# ------------------------------------------------------------- custom ops --

def _get_ops():
    if "ops" in _CACHE:
        return _CACHE["ops"]
    import concourse.dve_ops as dve_ops_mod
    from concourse.dve_ops import DveOp, _COMPILE_CACHE, get_dve_sub_opcode
    from concourse.dve_spec import (
        Spec, Src0, Src1, Zero, maxx, lower, _has_src1, Bin, AluOp,
    )
    from concourse.dve_uop import (
        AluInp, DelayInp, InpSel, OutPath, OutSel, Trigger, UopConfig,
        UopDpConfig, ENABLE, N_STAGES, DveOpSpec,
    )

    def _register(name, spec, build_fn):
        for op in dve_ops_mod.OPS:
            if op.name == name:
                return op
        if name not in dve_ops_mod._SUB_OPCODE_FOR_NAME:
            row = max(dve_ops_mod._SUB_OPCODE_FOR_NAME.values()) + 1
            assert row < 0x20
            dve_ops_mod._SUB_OPCODE_FOR_NAME[name] = row
        shas = {}
        for ver in ("v3", "v4"):
            try:
                s = build_fn(ver)
                shas[ver] = s.sha(ver)
                _COMPILE_CACHE[(name, ver)] = s
            except Exception:
                pass
        assert shas, f"{name}: no version compiled"
        op = DveOp(name, spec, False, shas)
        dve_ops_mod.OPS.append(op)
        dve_ops_mod.CUSTOM_DVE_SPECS[name] = spec
        return op

    # ---- super op: out = max(-in0, in1); accum_out = rowmax(-in0) ----
    def _super_ref(in0, in1, s0, s1, imm2):
        neg = -np.asarray(in0, np.float32)
        out = np.maximum(neg, np.asarray(in1, np.float32))
        acc = neg.reshape(neg.shape[0], -1).max(axis=-1, keepdims=True)
        return out, acc

    SUPER_SPEC = Spec(
        body=maxx(Bin(AluOp.SUBTRACT, Zero, Src0), Src1),
        accum=maxx,
        reference=_super_ref,
    )

    def _build_super(ver):
        name = "CHAMF_SUPER_ANT"
        uops = lower(SUPER_SPEC, ver=ver)
        assert len(uops) == 2
        steady = uops[1]
        dp = steady.datapath_config
        assert dp[0].op == AluOp.SUBTRACT
        assert dp[1].op == AluOp.MAX
        assert (dp[2].op == AluOp.MAX
                and dp[2].alu_src0 == AluInp.CURR_ALU_OUT
                and dp[2].alu_src1 == AluInp.PREV_ALU_OUT)
        used = set()
        for i, en in enumerate(steady.inp_enable[1:7]):
            if en:
                used.add(i)
        for blk in dp:
            for c in range(6):
                if blk.delay_enable[c]:
                    used.add(c)
        free = [c for c in range(6) if c not in used]
        assert free
        c = free[0]
        # rewire the accumulator to reduce -in0 (stage0's output) instead
        # of the body root
        dp[1].enable_delay_from_src(DelayInp.PREV_ALU_OUT, c)
        dp[2].alu_src1 = AluInp(int(AluInp.PREV_DELAY_0) + c)
        steady.validate(ver)
        return DveOpSpec(name=name, opcode=get_dve_sub_opcode(name),
                         uops=uops, rd1_en=_has_src1(SUPER_SPEC))

    # ---- packed rowmax: accum_out(bf16) = max over both bf16 streams ----
    def _rmax2_ref(in0, in1, s0, s1, imm2):
        a = np.asarray(in0, np.float32)
        b = np.asarray(in1, np.float32)
        body = np.maximum(a, b)
        acc = body.reshape(body.shape[0], -1).max(axis=-1, keepdims=True)
        return body, acc

    RMAX2_SPEC = Spec(body=maxx(Src0, Src1), accum=maxx, reference=_rmax2_ref)

    def _mk_2x(ver):
        n_stages = N_STAGES[ver]
        PD = lambda c: AluInp(int(AluInp.PREV_DELAY_0) + c)
        u = UopConfig()
        u.enable_input(InpSel.SRC_0, 0)
        u.enable_input(InpSel.SRC_1, 1)
        u.enable_input(InpSel.SRC_0_HI, 2)
        u.enable_input(InpSel.SRC_1_HI, 3)
        dp = [UopDpConfig() for _ in range(n_stages)]
        dp[0].enable_alu(AluOp.MAX, AluInp.PREV_ALU_OUT, PD(0))
        dp[0].pass_through_delay(1, 2)
        dp[1].enable_alu(AluOp.MAX, PD(1), PD(2))
        dp[1].enable_delay_from_src(DelayInp.PREV_ALU_OUT, 0)
        dp[2].enable_alu(AluOp.MAX, AluInp.PREV_ALU_OUT, PD(0))
        dp[2].pass_through_delay(0)
        dp[2].enable_delay_from_src(DelayInp.PREV_ALU_OUT, 1)
        dp[3].enable_alu(AluOp.MAX, AluInp.CURR_ALU_OUT, AluInp.PREV_ALU_OUT)
        dp[3].alu_out_a_enable = ENABLE
        dp[3].pass_through_delay(0, 1)
        for k in range(4, n_stages):
            dp[k].enable_alu(AluOp.BYPASS, AluInp.PREV_ALU_OUT,
                             AluInp.PREV_ALU_OUT)
            dp[k].alu_out_a_enable = ENABLE
            dp[k].pass_through_delay(0, 1)
        u.datapath_config = dp
        u.accum_enabled = ENABLE
        u.enable_output(OutSel.DELAY_0, OutPath.WR0_LO)
        u.enable_output(OutSel.DELAY_1, OutPath.WR0_HI)
        u.require_inp0 = 1
        u.require_inp1 = 1
        u.trigger = (Trigger.SRC_TENSOR_DONE, Trigger.NONE, Trigger.NONE)
        u.next_uop = (0, 0, 0)

        s = UopConfig()
        sdp = [UopDpConfig() for _ in range(n_stages)]
        s.enable_input(InpSel.MAX_NEG, 1)
        for k in range(0, 3):
            sdp[k].pass_through_delay(0)
        sdp[3].enable_alu(AluOp.BYPASS, PD(0), PD(0))
        sdp[3].alu_out_a_enable = ENABLE
        for k in range(4, n_stages):
            sdp[k].enable_alu(AluOp.BYPASS, AluInp.PREV_ALU_OUT,
                              AluInp.PREV_ALU_OUT)
            sdp[k].alu_out_a_enable = ENABLE
        s.datapath_config = sdp
        s.accum_enabled = ENABLE
        s.trigger = (Trigger.COUNT, Trigger.NONE, Trigger.NONE)
        s.repeat_count = 1
        s.next_uop = (1, 0, 0)
        return [s, u]

    def _build_rmax2(ver):
        name = "CHAMF_RMAX2_ANT"
        uops = lower(RMAX2_SPEC, ver=ver)
        assert len(uops) == 2
        return DveOpSpec(name=name, opcode=get_dve_sub_opcode(name),
                         uops=uops, rd1_en=_has_src1(RMAX2_SPEC),
                         uops_2x=_mk_2x(ver), perf_max=1)

    SUPER = _register("CHAMF_SUPER_ANT", SUPER_SPEC, _build_super)
    RMAX2 = _register("CHAMF_RMAX2_ANT", RMAX2_SPEC, _build_rmax2)
    _CACHE["ops"] = (SUPER, RMAX2)
    return _CACHE["ops"]


# ---------------------------------------------------------------- host prep --

def _split3(v):
    h = v.astype(BF)
    r = v - h.astype(np.float64)
    m = r.astype(BF)
    r2 = r - m.astype(np.float64)
    l = r2.astype(BF)
    return h, m, l


def _build_tabs(X, Y):
    """lhsT (24, N) bf16 and rhs (24, M) bf16 with
    (lhsT.T @ rhs)[n, m] ~= |X_n|^2 - 2<X_n, Y_m> + |Y_m|^2."""
    lt = np.empty((K, X.shape[0]), BF)
    rt = np.empty((K, Y.shape[0]), BF)
    Xd = X.astype(np.float64)
    Yd = -2.0 * Y.astype(np.float64)
    row = 0
    for c in range(C):
        Xh, Xm, Xl = _split3(Xd[:, c])
        Yh, Ym, Yl = _split3(Yd[:, c])
        for a, b in ((Xh, Yh), (Xh, Ym), (Xm, Yh), (Xm, Ym), (Xh, Yl), (Xl, Yh)):
            lt[row] = a
            rt[row] = b
            row += 1
    q = np.sum(Y.astype(np.float64) ** 2, axis=1)
    qh, qm, ql = _split3(q)
    ones_n = np.ones(X.shape[0], BF)
    for qq in (qh, qm, ql):
        lt[row] = ones_n
        rt[row] = qq
        row += 1
    p = np.sum(X.astype(np.float64) ** 2, axis=1)
    ph, pm, pl = _split3(p)
    ones_m = np.ones(Y.shape[0], BF)
    for pp in (ph, pm, pl):
        lt[row] = pp
        rt[row] = ones_m
        row += 1
    assert row == K
    return lt, rt


# ------------------------------------------------------------- device build --

def _build_nc(reps=1):
    key = ("nc", reps)
    if key in _CACHE:
        return _CACHE[key]
    import concourse.bacc as bacc
    import concourse.mybir as mybir
    from concourse.tile import TileContext

    SUPER, RMAX2 = _get_ops()
    f32 = mybir.dt.float32
    bf16 = mybir.dt.bfloat16
    alu = mybir.AluOpType
    Copy = mybir.ActivationFunctionType.Copy

    nc = bacc.Bacc(None)
    ltab = nc.dram_tensor("ltab", [BPC, K, N], bf16, kind="ExternalInput")
    rtab = nc.dram_tensor("rtab", [BPC, K, M], bf16, kind="ExternalInput")
    rowf = nc.dram_tensor("rowf", [128, BPC * SLOTF], f32, kind="ExternalOutput")
    rowb = nc.dram_tensor("rowb", [128, BPC * SLOTB], bf16, kind="ExternalOutput")
    n_acc = 1 + (1 if N_D else 0)
    rcol = nc.dram_tensor("rcol", [128, BPC * n_acc * M], bf16,
                          kind="ExternalOutput")
    gcol = nc.dram_tensor("gcol", [BPC, max(N_G, 1), M], bf16,
                          kind="ExternalOutput")

    with TileContext(nc) as tc:
        with (
            tc.tile_pool(name="tabs", bufs=1) as tabs,
            tc.tile_pool(name="psum", bufs=4, space="PSUM") as psum,
            tc.tile_pool(name="qp", bufs=12) as qp,
            tc.tile_pool(name="stg", bufs=4) as stg,
            tc.tile_pool(name="res", bufs=1) as res,
        ):
            neginf = res.tile([128, CHW], bf16, tag="neginf", name="neginf")
            nc.vector.memset(neginf[:, :], NEG_INIT)
            dummies = [res.tile([128, CHW], bf16, tag=f"dum{d}", name=f"dum{d}")
                       for d in range(3)]
            Rs = {}      # (b, acc) -> [tileA, tileB] ping-pong [128, M] bf16
            rafs = {}
            rabs = {}
            lts, rts = {}, {}
            for b in range(BPC):
                for a, an in ((0, "s"), (1, "d")):
                    if a == 1 and N_D == 0:
                        continue
                    Rs[(b, a)] = [
                        res.tile([128, M], bf16, tag=f"R{an}{b}{x}",
                                 name=f"R{an}{b}{x}") for x in "ab"]
                rafs[b] = res.tile([128, SLOTF], f32, tag=f"raf{b}",
                                   name=f"raf{b}")
                rabs[b] = res.tile([128, SLOTB], bf16, tag=f"rab{b}",
                                   name=f"rab{b}")
                nc.vector.memset(rabs[b][:, :], 0.0)
                lts[b] = tabs.tile([K, N], bf16, tag=f"lt{b}", name=f"lt{b}")
                rts[b] = tabs.tile([K, M], bf16, tag=f"rt{b}", name=f"rt{b}")
                nc.sync.dma_start(out=lts[b][:, :], in_=ltab[b])
                nc.sync.dma_start(out=rts[b][:, :], in_=rtab[b])

            for _rep in range(reps):
                for b in range(BPC):
                    lt, rt = lts[b], rts[b]
                    chain = {}   # (acc, c) -> update count
                    gp_cnt = 0
                    si = 0
                    ai = 0
                    dum_i = 0
                    for t in range(NT):
                        path = PATTERN[t]
                        ltT = lt[:, t * 128:(t + 1) * 128]
                        if path == "S":
                            for c in range(NCH):
                                pt = psum.tile([128, CHW], f32, tag="pt",
                                               name="pt")
                                base = c * CHW
                                for j in range(CHW // 512):
                                    nc.tensor.matmul(
                                        pt[:, j * 512:(j + 1) * 512], ltT,
                                        rt[:, base + j * 512:base + (j + 1) * 512],
                                        start=True, stop=True)
                                kcnt = chain.get((0, c), 0)
                                bufs = Rs[(b, 0)]
                                sl = slice(c * CHW, (c + 1) * CHW)
                                in1 = (neginf[:, :] if kcnt == 0
                                       else bufs[kcnt % 2][:, sl])
                                slot = NCH * si + c
                                nc.vector._custom_dve(
                                    SUPER,
                                    out=bufs[(kcnt + 1) % 2][:, sl],
                                    in0=pt[:, :],
                                    in1=in1,
                                    accum_out=rafs[b][:, slot:slot + 1],
                                )
                                chain[(0, c)] = kcnt + 1
                            si += 1
                        else:
                            qts = []
                            for c in range(NCH):
                                pt = psum.tile([128, CHW], f32, tag="pt",
                                               name="pt")
                                base = c * CHW
                                for j in range(CHW // 512):
                                    nc.tensor.matmul(
                                        pt[:, j * 512:(j + 1) * 512], ltT,
                                        rt[:, base + j * 512:base + (j + 1) * 512],
                                        start=True, stop=True)
                                q = qp.tile([128, CHW], bf16, tag="q", name="q")
                                nc.scalar.activation(out=q[:, :], in_=pt[:, :],
                                                     func=Copy, scale=-1.0)
                                qts.append(q)
                                if path == "D":
                                    kcnt = chain.get((1, c), 0)
                                    bufs = Rs[(b, 1)]
                                    sl = slice(c * CHW, (c + 1) * CHW)
                                    if kcnt == 0:
                                        nc.vector.tensor_copy(
                                            out=bufs[1][:, sl], in_=q[:, :])
                                    else:
                                        nc.vector.tensor_tensor(
                                            out=bufs[(kcnt + 1) % 2][:, sl],
                                            in0=bufs[kcnt % 2][:, sl],
                                            in1=q[:, :], op=alu.max)
                                    chain[(1, c)] = kcnt + 1
                                else:  # G
                                    if c == 0:
                                        gst = stg.tile([1, M], bf16, tag="st",
                                                       name="gst")
                                    nc.gpsimd.tensor_reduce(
                                        out=gst[0:1, c * CHW:(c + 1) * CHW],
                                        in_=q[:, :],
                                        axis=mybir.AxisListType.C, op=alu.max)
                                    if c == NCH - 1:
                                        nc.sync.dma_start(
                                            out=gcol[b, gp_cnt:gp_cnt + 1, :],
                                            in_=gst[0:1, :])
                                        gp_cnt += 1
                            for pi in range(NCH // 2):
                                slot = 2 * ((NCH // 2) * ai + pi)
                                bi = nc.vector._custom_dve(
                                    RMAX2,
                                    out=dummies[dum_i][:, :],
                                    in0=qts[2 * pi][:, :],
                                    in1=qts[2 * pi + 1][:, :],
                                    accum_out=rabs[b][:, slot:slot + 1],
                                )
                                bi.ins.perf_max = 1
                                dum_i = (dum_i + 1) % len(dummies)
                            ai += 1
                    # finals for this batch/rep
                    assert N_G == 0 or gp_cnt == N_G
                    for a in range(2):
                        if (a, 0) not in chain:
                            continue
                        for c in range(NCH):
                            n_upd = chain[(a, c)]
                            fin = Rs[(b, a)][n_upd % 2]
                            sl = slice(c * CHW, (c + 1) * CHW)
                            col0 = (b * n_acc + a) * M + c * CHW
                            nc.sync.dma_start(
                                out=rcol[:, col0:col0 + CHW], in_=fin[:, sl])
                    nc.sync.dma_start(
                        out=rowf[:, b * SLOTF:(b + 1) * SLOTF],
                        in_=rafs[b][:, :])
                    nc.sync.dma_start(
                        out=rowb[:, b * SLOTB:(b + 1) * SLOTB],
                        in_=rabs[b][:, :])
    nc.compile()
    _CACHE[key] = nc
    return nc


# -------------------------------------------------------------------- entry --

def _prepare_inputs(pred, target):
    ltabs = np.empty((NCORES, BPC, K, N), BF)
    rtabs = np.empty((NCORES, BPC, K, M), BF)
    for core in range(NCORES):
        for lb in range(BPC):
            bidx = core * BPC + lb
            lt, rt = _build_tabs(pred[bidx], target[bidx])
            ltabs[core, lb] = lt
            rtabs[core, lb] = rt
    return ltabs, rtabs


def _postprocess(results):
    n_acc = 1 + (1 if N_D else 0)
    losses = []
    for core in range(NCORES):
        out = results[core]
        rowf = np.asarray(out["rowf"], np.float32)
        rowb = np.asarray(out["rowb"], np.float32)
        rcol = np.asarray(out["rcol"], np.float32)
        gcol = np.asarray(out["gcol"], np.float32).reshape(BPC, max(N_G, 1), M)
        for b in range(BPC):
            rf = rowf[:, b * SLOTF:(b + 1) * SLOTF]
            rb = rowb[:, b * SLOTB:(b + 1) * SLOTB]
            si = 0
            ai = 0
            rows = np.empty((NT, 128), np.float64)
            for t in range(NT):
                if PATTERN[t] == "S":
                    sl = rf[:, NCH * si:NCH * (si + 1)]
                    rows[t] = -sl.max(axis=1)
                    si += 1
                else:
                    cols = [2 * ((NCH // 2) * ai + pi) for pi in range(NCH // 2)]
                    rows[t] = -rb[:, cols].max(axis=1)
                    ai += 1
            rowmin = rows.reshape(-1)
            cand = rcol[:, (b * n_acc + 0) * M:(b * n_acc + 1) * M].max(axis=0)
            if N_D:
                cd = rcol[:, (b * n_acc + 1) * M:(b * n_acc + 2) * M].max(axis=0)
                cand = np.maximum(cand, cd)
            if N_G:
                cand = np.maximum(cand, gcol[b].max(axis=0))
            colmin = -cand
            losses.append(rowmin.mean() + colmin.mean())
    return np.float32(np.mean(losses))


def _run(pred, target, trace=False):
    from concourse.bass_utils import run_bass_kernel_spmd

    pred = np.asarray(pred, dtype=np.float32)
    target = np.asarray(target, dtype=np.float32)
    assert pred.shape == (B, N, C) and target.shape == (B, M, C)
    ltabs, rtabs = _prepare_inputs(pred, target)
    nc = _build_nc()
    in_maps = [{"ltab": ltabs[c], "rtab": rtabs[c]} for c in range(NCORES)]
    res = run_bass_kernel_spmd(nc, in_maps, core_ids=list(range(NCORES)),
                               trace=trace)
    return _postprocess(res.results), res


def kernel(pred, target):
    loss, _ = _run(pred, target, trace=False)
    return loss


# revision 8
# speedup vs baseline: 106.3053x; 106.3053x over previous
"""Chamfer L2 distance kernel for 8 Trainium2 NeuronCores.

Strategy (data-parallel over batch, 2 batches/core):
  For each batch and each direction (pred->target, target->pred) the device
  computes rowmin[n] = min_m H[n, m] where H = -2<x_n, y_m> + |y_m|^2 via
  K=21 bf16 matmuls (an exact hi/mid/lo bf16 decomposition of the fp32
  inputs, error ~1e-7 absolute) and a fused custom DVE min/min-reduce that
  consumes two fresh 1024-wide PSUM/SBUF tiles per pass.  The host adds the
  partition-side norms |x_n|^2 and finishes the means in fp64.

Self-contained: hardcodes B=16, N=M=4096, C=3, 8 cores.
"""

import numpy as np
import ml_dtypes

BF = ml_dtypes.bfloat16
B, N, M, C = 16, 4096, 4096, 3
NCORES = 8
BPC = B // NCORES          # batches per core
NU = BPC * 2               # (batch, orientation) units per core
K = 21                     # contraction rows (18 product terms + 3 norm rows)
NT = N // 128              # n-tiles per unit
SLOTS = NU * NT * 2        # accum slots (2 m-halves per n-tile)

_CACHE = {}


# ---------------------------------------------------------------- host prep --

def _split3(v):
    """Exact-ish 3-way bf16 decomposition: h + m + l = v + O(2^-27 |v|)."""
    h = v.astype(BF)
    r = v - h.astype(np.float64)
    m = r.astype(BF)
    r2 = r - m.astype(np.float64)
    l = r2.astype(BF)
    return h, m, l


def _build_tabs(X, Y):
    """X: (N,3) partition side, Y: (M,3) free side.
    Returns lhsT (21, N) bf16 and rhs (21, M) bf16 such that
    (lhsT.T @ rhs)[n, m] ~= -2<X_n, Y_m> + |Y_m|^2 to ~1e-7 absolute."""
    lt = np.empty((K, X.shape[0]), BF)
    rt = np.empty((K, Y.shape[0]), BF)
    Xd = X.astype(np.float64)
    Yd = -2.0 * Y.astype(np.float64)
    row = 0
    for c in range(C):
        Xh, Xm, Xl = _split3(Xd[:, c])
        Yh, Ym, Yl = _split3(Yd[:, c])
        for a, b in ((Xh, Yh), (Xh, Ym), (Xm, Yh), (Xm, Ym), (Xh, Yl), (Xl, Yh)):
            lt[row] = a
            rt[row] = b
            row += 1
    q = np.sum(Y.astype(np.float64) ** 2, axis=1)
    qh, qm, ql = _split3(q)
    ones = np.ones(X.shape[0], BF)
    for qq in (qh, qm, ql):
        lt[row] = ones
        rt[row] = qq
        row += 1
    assert row == K
    return lt, rt


# ------------------------------------------------------------- device build --

def _get_min_min_op():
    if "op" in _CACHE:
        return _CACHE["op"]
    import concourse.dve_ops as dve_ops_mod
    from concourse.dve_ops import DveOp
    from concourse.dve_spec import Spec, Src0, Src1, C0, minn, lower, _has_src1
    from concourse.dve_uop import DveOpSpec

    name = "CHAMFER_MIN_MIN_ANT"
    for op in dve_ops_mod.OPS:
        if op.name == name:
            _CACHE["op"] = op
            return op
    spec = Spec(
        body=minn(Src0, Src1),
        accum=minn,
        accum_init=C0,
        reference=lambda in0, in1, s0, s1, imm2: (
            (b := np.minimum(in0.astype(np.float32), in1.astype(np.float32))),
            np.minimum(
                b.reshape(b.shape[0], -1).min(axis=-1, keepdims=True),
                np.asarray(s0, np.float32).reshape(-1, 1),
            ),
        ),
    )
    if name not in dve_ops_mod._SUB_OPCODE_FOR_NAME:
        row = max(dve_ops_mod._SUB_OPCODE_FOR_NAME.values()) + 1
        assert row < 0x20
        dve_ops_mod._SUB_OPCODE_FOR_NAME[name] = row
    shas = {}
    for ver in ("v3", "v4"):
        try:
            s = DveOpSpec(
                name=name,
                opcode=dve_ops_mod.get_dve_sub_opcode(name),
                uops=lower(spec, ver=ver),
                rd1_en=_has_src1(spec),
            )
            shas[ver] = s.sha(ver)
        except Exception:
            pass
    op = DveOp(name, spec, False, shas)
    dve_ops_mod.OPS.append(op)
    dve_ops_mod.CUSTOM_DVE_SPECS[name] = spec
    _CACHE["op"] = op
    return op


def _build_nc(reps=1):
    key = ("nc", reps)
    if key in _CACHE:
        return _CACHE[key]
    import concourse.bacc as bacc
    import concourse.mybir as mybir
    from concourse.tile import TileContext

    MIN_MIN = _get_min_min_op()
    f32 = mybir.dt.float32
    bf16 = mybir.dt.bfloat16

    nc = bacc.Bacc(None)
    ltab = nc.dram_tensor("ltab", [NU, K, N], bf16, kind="ExternalInput")
    rtab = nc.dram_tensor("rtab", [NU, K, M], bf16, kind="ExternalInput")
    outt = nc.dram_tensor("out", [128, SLOTS], f32, kind="ExternalOutput")

    with TileContext(nc) as tc:
        with (
            tc.tile_pool(name="stage", bufs=2) as stage,
            tc.tile_pool(name="psum", bufs=2, space="PSUM") as psum,
            tc.tile_pool(name="cpp", bufs=3) as cpp,
            tc.tile_pool(name="res", bufs=1) as res,
        ):
            raw = res.tile([128, SLOTS], f32, tag="raw")
            # rotate the discarded broadcast-out target so consecutive DVE ops
            # have no WAW on the same tile
            dummies = [res.tile([128, 1], f32, tag=f"dummy{d}", name=f"dummy{d}")
                       for d in range(4)]
            for _rep in range(reps):
              for u in range(NU):
                  lt = stage.tile([K, N], bf16, tag="lt")
                  rt = stage.tile([K, M], bf16, tag="rt")
                  nc.sync.dma_start(out=lt[:, :], in_=ltab[u])
                  nc.sync.dma_start(out=rt[:, :], in_=rtab[u])
                  for i in range(NT):
                      ltT = lt[:, i * 128:(i + 1) * 128]
                      for h in range(2):
                          pa = psum.tile([128, 1024], f32, tag="pa")
                          pb = psum.tile([128, 1024], f32, tag="pb")
                          base = h * 2048
                          nc.tensor.matmul(pb[:, 0:512], ltT, rt[:, base + 1024:base + 1536],
                                           start=True, stop=True)
                          nc.tensor.matmul(pb[:, 512:1024], ltT, rt[:, base + 1536:base + 2048],
                                           start=True, stop=True)
                          nc.tensor.matmul(pa[:, 0:512], ltT, rt[:, base:base + 512],
                                           start=True, stop=True)
                          nc.tensor.matmul(pa[:, 512:1024], ltT, rt[:, base + 512:base + 1024],
                                           start=True, stop=True)
                          cp = cpp.tile([128, 1024], f32, tag="cp")
                          nc.scalar.copy(out=cp[:, :], in_=pb[:, :])
                          slot = (u * NT + i) * 2 + h
                          nc.vector._custom_dve(
                              MIN_MIN,
                              out=dummies[slot % 4].broadcast_to(pa[:, :].shape),
                              in0=pa[:, :],
                              in1=cp[:, :],
                              s0=1.0e30,
                              accum_out=raw[:, slot:slot + 1],
                          )
            nc.sync.dma_start(out=outt[:, :], in_=raw[:, :])
    nc.compile()
    _CACHE[key] = nc
    return nc


# -------------------------------------------------------------------- entry --

def _prepare_inputs(pred, target):
    ltabs = np.empty((NCORES, NU, K, N), BF)
    rtabs = np.empty((NCORES, NU, K, M), BF)
    for core in range(NCORES):
        for lb in range(BPC):
            b = core * BPC + lb
            for o in range(2):
                X = pred[b] if o == 0 else target[b]
                Y = target[b] if o == 0 else pred[b]
                lt, rt = _build_tabs(X, Y)
                u = lb * 2 + o
                ltabs[core, u] = lt
                rtabs[core, u] = rt
    return ltabs, rtabs


def _postprocess(results, pred, target):
    losses = []
    for core in range(NCORES):
        out = np.asarray(results[core]["out"])  # (128, SLOTS)
        for lb in range(BPC):
            b = core * BPC + lb
            total = 0.0
            for o in range(2):
                u = lb * 2 + o
                sl = out[:, u * (NT * 2):(u + 1) * (NT * 2)]
                rowmin = sl.reshape(128, NT, 2).min(axis=2)      # (p, i)
                rowmin = rowmin.T.reshape(-1)                     # n = i*128 + p
                X = pred[b] if o == 0 else target[b]
                s2 = np.sum(X.astype(np.float64) ** 2, axis=1)
                total += (s2 + rowmin).mean()
            losses.append(total)
    return np.float32(np.mean(losses))


def _run(pred, target, trace=False):
    from concourse.bass_utils import run_bass_kernel_spmd

    pred = np.asarray(pred, dtype=np.float32)
    target = np.asarray(target, dtype=np.float32)
    assert pred.shape == (B, N, C) and target.shape == (B, M, C)
    ltabs, rtabs = _prepare_inputs(pred, target)
    nc = _build_nc()
    in_maps = [{"ltab": ltabs[c], "rtab": rtabs[c]} for c in range(NCORES)]
    res = run_bass_kernel_spmd(nc, in_maps, core_ids=list(range(NCORES)), trace=trace)
    return _postprocess(res.results, pred, target), res


def kernel(pred, target):
    loss, _ = _run(pred, target, trace=False)
    return loss



# revision 9
# speedup vs baseline: 130.9873x; 1.2322x over previous
"""Chamfer L2 distance kernel for 8 Trainium2 NeuronCores.

Strategy (data-parallel over batch, 2 batches/core):
  For each batch and each direction (pred->target, target->pred) the device
  computes rowmin[n] = min_m H[n, m] where H = -2<x_n, y_m> + |y_m|^2 via
  K=21 bf16 matmuls (an exact hi/mid/lo bf16 decomposition of the fp32
  inputs, error ~1e-7 absolute) and a fused custom DVE min/min-reduce that
  consumes two fresh 1024-wide PSUM/SBUF tiles per pass.  The host adds the
  partition-side norms |x_n|^2 and finishes the means in fp64.

Self-contained: hardcodes B=16, N=M=4096, C=3, 8 cores.
"""

import numpy as np
import ml_dtypes

BF = ml_dtypes.bfloat16
B, N, M, C = 16, 4096, 4096, 3
NCORES = 8
BPC = B // NCORES          # batches per core
NU = BPC * 2               # (batch, orientation) units per core
K = 24                     # contraction rows (18 products + 3 |y|^2 + 3 |x|^2)
NT = N // 128              # n-tiles per unit
SLOTS = NU * NT * 2        # accum slots (2 m-halves per n-tile)

_CACHE = {}


# ---------------------------------------------------------------- host prep --

def _split3(v):
    """Exact-ish 3-way bf16 decomposition: h + m + l = v + O(2^-27 |v|)."""
    h = v.astype(BF)
    r = v - h.astype(np.float64)
    m = r.astype(BF)
    r2 = r - m.astype(np.float64)
    l = r2.astype(BF)
    return h, m, l


def _build_tabs(X, Y):
    """X: (N,3) partition side, Y: (M,3) free side.
    Returns lhsT (21, N) bf16 and rhs (21, M) bf16 such that
    (lhsT.T @ rhs)[n, m] ~= -2<X_n, Y_m> + |Y_m|^2 to ~1e-7 absolute."""
    lt = np.empty((K, X.shape[0]), BF)
    rt = np.empty((K, Y.shape[0]), BF)
    Xd = X.astype(np.float64)
    Yd = -2.0 * Y.astype(np.float64)
    row = 0
    for c in range(C):
        Xh, Xm, Xl = _split3(Xd[:, c])
        Yh, Ym, Yl = _split3(Yd[:, c])
        for a, b in ((Xh, Yh), (Xh, Ym), (Xm, Yh), (Xm, Ym), (Xh, Yl), (Xl, Yh)):
            lt[row] = a
            rt[row] = b
            row += 1
    q = np.sum(Y.astype(np.float64) ** 2, axis=1)
    qh, qm, ql = _split3(q)
    ones = np.ones(X.shape[0], BF)
    for qq in (qh, qm, ql):
        lt[row] = ones
        rt[row] = qq
        row += 1
    p = np.sum(X.astype(np.float64) ** 2, axis=1)
    ph, pm, pl = _split3(p)
    ones_m = np.ones(Y.shape[0], BF)
    for pp in (ph, pm, pl):
        lt[row] = pp
        rt[row] = ones_m
        row += 1
    assert row == K
    return lt, rt


# ------------------------------------------------------------- device build --

def _get_min_min_op():
    if "op" in _CACHE:
        return _CACHE["op"]
    import concourse.dve_ops as dve_ops_mod
    from concourse.dve_ops import DveOp
    from concourse.dve_spec import Spec, Src0, Src1, C0, minn, lower, _has_src1
    from concourse.dve_uop import DveOpSpec

    name = "CHAMFER_MIN_MIN_ANT"
    for op in dve_ops_mod.OPS:
        if op.name == name:
            _CACHE["op"] = op
            return op
    spec = Spec(
        body=minn(Src0, Src1),
        accum=minn,
        accum_init=C0,
        reference=lambda in0, in1, s0, s1, imm2: (
            (b := np.minimum(in0.astype(np.float32), in1.astype(np.float32))),
            np.minimum(
                b.reshape(b.shape[0], -1).min(axis=-1, keepdims=True),
                np.asarray(s0, np.float32).reshape(-1, 1),
            ),
        ),
    )
    if name not in dve_ops_mod._SUB_OPCODE_FOR_NAME:
        row = max(dve_ops_mod._SUB_OPCODE_FOR_NAME.values()) + 1
        assert row < 0x20
        dve_ops_mod._SUB_OPCODE_FOR_NAME[name] = row
    shas = {}
    for ver in ("v3", "v4"):
        try:
            s = DveOpSpec(
                name=name,
                opcode=dve_ops_mod.get_dve_sub_opcode(name),
                uops=lower(spec, ver=ver),
                rd1_en=_has_src1(spec),
            )
            shas[ver] = s.sha(ver)
        except Exception:
            pass
    op = DveOp(name, spec, False, shas)
    dve_ops_mod.OPS.append(op)
    dve_ops_mod.CUSTOM_DVE_SPECS[name] = spec
    _CACHE["op"] = op
    return op


def _get_rmax2_op():
    if "rmax2" in _CACHE:
        return _CACHE["rmax2"]
    import concourse.dve_ops as dve_ops_mod
    from concourse.dve_ops import DveOp, _COMPILE_CACHE, get_dve_sub_opcode
    from concourse.dve_spec import Spec, Src0, Src1, maxx, lower, _has_src1
    from concourse.dve_uop import (
        AluInp, DelayInp, InpSel, OutPath, OutSel, Trigger, UopConfig,
        UopDpConfig, ENABLE, N_STAGES, DveOpSpec,
    )
    from concourse.dve_spec import AluOp

    name = "CHAMF_RMAX2_ANT"
    for op in dve_ops_mod.OPS:
        if op.name == name:
            _CACHE["rmax2"] = op
            return op

    def _ref(in0, in1, s0, s1, imm2):
        a = np.asarray(in0, np.float32)
        b = np.asarray(in1, np.float32)
        body = np.maximum(a, b)
        return body, body.reshape(body.shape[0], -1).max(axis=-1, keepdims=True)

    spec = Spec(body=maxx(Src0, Src1), accum=maxx, reference=_ref)

    def _mk_2x(ver):
        n_stages = N_STAGES[ver]
        PD = lambda c: AluInp(int(AluInp.PREV_DELAY_0) + c)
        u = UopConfig()
        u.enable_input(InpSel.SRC_0, 0)
        u.enable_input(InpSel.SRC_1, 1)
        u.enable_input(InpSel.SRC_0_HI, 2)
        u.enable_input(InpSel.SRC_1_HI, 3)
        dp = [UopDpConfig() for _ in range(n_stages)]
        dp[0].enable_alu(AluOp.MAX, AluInp.PREV_ALU_OUT, PD(0))
        dp[0].pass_through_delay(1, 2)
        dp[1].enable_alu(AluOp.MAX, PD(1), PD(2))
        dp[1].enable_delay_from_src(DelayInp.PREV_ALU_OUT, 0)
        dp[2].enable_alu(AluOp.MAX, AluInp.PREV_ALU_OUT, PD(0))
        dp[2].pass_through_delay(0)
        dp[2].enable_delay_from_src(DelayInp.PREV_ALU_OUT, 1)
        dp[3].enable_alu(AluOp.MAX, AluInp.CURR_ALU_OUT, AluInp.PREV_ALU_OUT)
        dp[3].alu_out_a_enable = ENABLE
        dp[3].pass_through_delay(0, 1)
        for k in range(4, n_stages):
            dp[k].enable_alu(AluOp.BYPASS, AluInp.PREV_ALU_OUT,
                             AluInp.PREV_ALU_OUT)
            dp[k].alu_out_a_enable = ENABLE
            dp[k].pass_through_delay(0, 1)
        u.datapath_config = dp
        u.accum_enabled = ENABLE
        u.enable_output(OutSel.DELAY_0, OutPath.WR0_LO)
        u.enable_output(OutSel.DELAY_1, OutPath.WR0_HI)
        u.require_inp0 = 1
        u.require_inp1 = 1
        u.trigger = (Trigger.SRC_TENSOR_DONE, Trigger.NONE, Trigger.NONE)
        u.next_uop = (0, 0, 0)
        s = UopConfig()
        sdp = [UopDpConfig() for _ in range(n_stages)]
        s.enable_input(InpSel.MAX_NEG, 1)
        for k in range(0, 3):
            sdp[k].pass_through_delay(0)
        sdp[3].enable_alu(AluOp.BYPASS, PD(0), PD(0))
        sdp[3].alu_out_a_enable = ENABLE
        for k in range(4, n_stages):
            sdp[k].enable_alu(AluOp.BYPASS, AluInp.PREV_ALU_OUT,
                              AluInp.PREV_ALU_OUT)
            sdp[k].alu_out_a_enable = ENABLE
        s.datapath_config = sdp
        s.accum_enabled = ENABLE
        s.trigger = (Trigger.COUNT, Trigger.NONE, Trigger.NONE)
        s.repeat_count = 1
        s.next_uop = (1, 0, 0)
        return [s, u]

    if name not in dve_ops_mod._SUB_OPCODE_FOR_NAME:
        row = max(dve_ops_mod._SUB_OPCODE_FOR_NAME.values()) + 1
        assert row < 0x20
        dve_ops_mod._SUB_OPCODE_FOR_NAME[name] = row
    shas = {}
    for ver in ("v3", "v4"):
        try:
            s = DveOpSpec(name=name, opcode=get_dve_sub_opcode(name),
                          uops=lower(spec, ver=ver), rd1_en=_has_src1(spec),
                          uops_2x=_mk_2x(ver), perf_max=1)
            shas[ver] = s.sha(ver)
            _COMPILE_CACHE[(name, ver)] = s
        except Exception:
            pass
    op = DveOp(name, spec, False, shas)
    dve_ops_mod.OPS.append(op)
    dve_ops_mod.CUSTOM_DVE_SPECS[name] = spec
    _CACHE["rmax2"] = op
    return op


def _build_nc(reps=1):
    key = ("nc", reps)
    if key in _CACHE:
        return _CACHE[key]
    import concourse.bacc as bacc
    import concourse.mybir as mybir
    from concourse.tile import TileContext

    MIN_MIN = _get_min_min_op()
    RMAX2 = _get_rmax2_op()
    f32 = mybir.dt.float32
    bf16 = mybir.dt.bfloat16
    Copy = mybir.ActivationFunctionType.Copy

    nc = bacc.Bacc(None)
    ltab = nc.dram_tensor("ltab", [NU, K, N], bf16, kind="ExternalInput")
    rtab = nc.dram_tensor("rtab", [NU, K, M], bf16, kind="ExternalInput")
    outt = nc.dram_tensor("out", [128, SLOTS], f32, kind="ExternalOutput")
    outb = nc.dram_tensor("outb", [128, SLOTS], bf16, kind="ExternalOutput")

    with TileContext(nc) as tc:
        with (
            tc.tile_pool(name="stage", bufs=2) as stage,
            tc.tile_pool(name="psum", bufs=2, space="PSUM") as psum,
            tc.tile_pool(name="cpp", bufs=3) as cpp,
            tc.tile_pool(name="qpool", bufs=6) as qpool,
            tc.tile_pool(name="res", bufs=1) as res,
        ):
            raw = res.tile([128, SLOTS], f32, tag="raw", name="raw")
            rawb = res.tile([128, SLOTS], bf16, tag="rawb", name="rawb")
            nc.vector.memset(rawb[:, :], 0.0)
            nc.vector.memset(raw[:, :], 0.0)
            bdums = [res.tile([128, 1024], bf16, tag=f"bdum{d}",
                              name=f"bdum{d}") for d in range(3)]
            # rotate the discarded broadcast-out target so consecutive DVE ops
            # have no WAW on the same tile
            dummies = [res.tile([128, 1], f32, tag=f"dummy{d}", name=f"dummy{d}")
                       for d in range(4)]
            for _rep in range(reps):
              for u in range(NU):
                  lt = stage.tile([K, N], bf16, tag="lt")
                  rt = stage.tile([K, M], bf16, tag="rt")
                  nc.sync.dma_start(out=lt[:, :], in_=ltab[u])
                  nc.sync.dma_start(out=rt[:, :], in_=rtab[u])
                  for i in range(NT):
                      ltT = lt[:, i * 128:(i + 1) * 128]
                      for h in range(2):
                          pa = psum.tile([128, 1024], f32, tag="pa")
                          pb = psum.tile([128, 1024], f32, tag="pb")
                          base = h * 2048
                          nc.tensor.matmul(pb[:, 0:512], ltT, rt[:, base + 1024:base + 1536],
                                           start=True, stop=True)
                          nc.tensor.matmul(pb[:, 512:1024], ltT, rt[:, base + 1536:base + 2048],
                                           start=True, stop=True)
                          nc.tensor.matmul(pa[:, 0:512], ltT, rt[:, base:base + 512],
                                           start=True, stop=True)
                          nc.tensor.matmul(pa[:, 512:1024], ltT, rt[:, base + 512:base + 1024],
                                           start=True, stop=True)
                          slot = (u * NT + i) * 2 + h
                          if (i + h) % 2 == 0:
                              cp = cpp.tile([128, 1024], f32, tag="cp",
                                            name="cp")
                              nc.scalar.copy(out=cp[:, :], in_=pb[:, :])
                              nc.vector._custom_dve(
                                  MIN_MIN,
                                  out=dummies[slot % 4].broadcast_to(pa[:, :].shape),
                                  in0=pa[:, :],
                                  in1=cp[:, :],
                                  s0=1.0e30,
                                  accum_out=raw[:, slot:slot + 1],
                              )
                          else:
                              qa = qpool.tile([128, 1024], bf16, tag="qa",
                                              name="qa")
                              qb = qpool.tile([128, 1024], bf16, tag="qb",
                                              name="qb")
                              nc.scalar.activation(out=qa[:, :], in_=pa[:, :],
                                                   func=Copy, scale=-1.0)
                              nc.scalar.activation(out=qb[:, :], in_=pb[:, :],
                                                   func=Copy, scale=-1.0)
                              bslot = 2 * (slot // 2)
                              bi = nc.vector._custom_dve(
                                  RMAX2,
                                  out=bdums[slot % 3][:, :],
                                  in0=qa[:, :],
                                  in1=qb[:, :],
                                  accum_out=rawb[:, bslot:bslot + 1],
                              )
                              bi.ins.perf_max = 1
            nc.sync.dma_start(out=outt[:, :], in_=raw[:, :])
            nc.sync.dma_start(out=outb[:, :], in_=rawb[:, :])
    nc.compile()
    _CACHE[key] = nc
    return nc


# -------------------------------------------------------------------- entry --

def _prepare_inputs(pred, target):
    ltabs = np.empty((NCORES, NU, K, N), BF)
    rtabs = np.empty((NCORES, NU, K, M), BF)
    for core in range(NCORES):
        for lb in range(BPC):
            b = core * BPC + lb
            for o in range(2):
                X = pred[b] if o == 0 else target[b]
                Y = target[b] if o == 0 else pred[b]
                lt, rt = _build_tabs(X, Y)
                u = lb * 2 + o
                ltabs[core, u] = lt
                rtabs[core, u] = rt
    return ltabs, rtabs


def _postprocess(results, pred, target):
    losses = []
    for core in range(NCORES):
        out = np.asarray(results[core]["out"], np.float64)   # (128, SLOTS)
        outb = np.asarray(results[core]["outb"], np.float64)  # (128, SLOTS)
        # merge: D-chunks in `out` (value = min D), A-chunks in `outb`
        # (value = max(-D) at even bslot = 2*(slot//2), i.e. same slot since
        # A-chunks have odd (i+h) -> slot parity ... use explicit mask
        merged = np.empty_like(out)
        for u in range(NU):
            for i in range(NT):
                for h in range(2):
                    slot = (u * NT + i) * 2 + h
                    if (i + h) % 2 == 0:
                        merged[:, slot] = out[:, slot]
                    else:
                        merged[:, slot] = -outb[:, 2 * (slot // 2)]
        for lb in range(BPC):
            b = core * BPC + lb
            total = 0.0
            for o in range(2):
                u = lb * 2 + o
                sl = merged[:, u * (NT * 2):(u + 1) * (NT * 2)]
                rowmin = sl.reshape(128, NT, 2).min(axis=2)
                rowmin = rowmin.T.reshape(-1)
                total += rowmin.mean()
            losses.append(total)
    return np.float32(np.mean(losses))


def _run(pred, target, trace=False):
    from concourse.bass_utils import run_bass_kernel_spmd

    pred = np.asarray(pred, dtype=np.float32)
    target = np.asarray(target, dtype=np.float32)
    assert pred.shape == (B, N, C) and target.shape == (B, M, C)
    ltabs, rtabs = _prepare_inputs(pred, target)
    nc = _build_nc()
    in_maps = [{"ltab": ltabs[c], "rtab": rtabs[c]} for c in range(NCORES)]
    res = run_bass_kernel_spmd(nc, in_maps, core_ids=list(range(NCORES)), trace=trace)
    return _postprocess(res.results, pred, target), res


def kernel(pred, target):
    loss, _ = _run(pred, target, trace=False)
    return loss

